# revision 5
# baseline (speedup 1.0000x reference)
"""MoE MLP (top-2 of 8 experts) Trainium2 kernel.

Strategy: expert-parallel across the 8 NeuronCores. The host computes the
(cheap, tiny) top-2 gating exactly in fp32, gathers each expert's tokens into
a contiguous capacity-padded buffer, and core e runs expert e's two big
matmuls over its gathered tokens:

    out_e = g_e * (prelu(Wfc[e] @ xT_sel, 0.5)^2)^T-chain  (all on device)

Device layout keeps the contraction dim on SBUF partitions throughout:
  mm1: psum[h_blk(128), tok(512)] += wfcT[d, h_blk]^T @ xT[d, tok]
  act: a = prelu(psum, 0.5); a *= a      (ScalarE + VectorE, fp16 out)
  mm2: psum[tok(128), d(512)]  += a[h, tok]^T @ wprojT[h, d]
  evict: out = psum * gate[token]        (per-partition scale on ScalarE)

Host scatters per-expert rows back (each token appears in exactly 2 expert
lists) and sums - identical math to the reference's dense masked combine.

Matmul inputs are fp16 (values are O(1); fp32 PSUM accumulation), weights are
cached in SBUF once per core, tokens stream through in 512-wide chunks.
"""

import numpy as np
from contextlib import ExitStack

B, T, D, H, E = 4, 2048, 1024, 4096, 8
N = B * T
P = 128
CHUNK = 512


def _build_nc(C):
    """Build + compile the per-core Bass program for capacity C tokens.

    C must be a multiple of 128. Tokens stream in chunks of 512 plus one
    optional tail chunk of C % 512.
    """
    import concourse.bacc as bacc
    import concourse.tile as tile
    import concourse.mybir as mybir

    assert C % P == 0
    f16 = mybir.dt.float16
    f32 = mybir.dt.float32
    AF = mybir.ActivationFunctionType

    nc = bacc.Bacc(None, target_bir_lowering=False, debug=False)
    xT = nc.dram_tensor("xT", [D, C], f16, kind="ExternalInput")
    wfcT = nc.dram_tensor("wfcT", [D, H], f16, kind="ExternalInput")
    wprojT = nc.dram_tensor("wprojT", [H, D], f16, kind="ExternalInput")
    g = nc.dram_tensor("g", [P, C // P], f32, kind="ExternalInput")
    out = nc.dram_tensor("outp", [C, D], f32, kind="ExternalOutput")

    xT_v = xT.ap().rearrange("(ko p) c -> p ko c", p=P)          # [128, 8, C]
    wfcT_v = wfcT.ap().rearrange("(ko p) h -> p ko h", p=P)      # [128, 8, H]
    wprojT_v = wprojT.ap().rearrange("(ko p) d -> p ko d", p=P)  # [128, 32, D]
    out_v = out.ap().rearrange("(c p) d -> p c d", p=P)          # [128, C//128, D]

    KD = D // P          # 8  k-subtiles for mm1
    KH = H // P          # 32 k-subtiles for mm2 (and h-blocks of mm1 output)
    DN = D // CHUNK      # 2 output-column blocks

    chunks = [CHUNK] * (C // CHUNK)
    if C % CHUNK:
        chunks.append(C % CHUNK)

    with tile.TileContext(nc) as tc:
        with ExitStack() as ctx:
            const = ctx.enter_context(tc.tile_pool(name="const", bufs=1))
            xpool = ctx.enter_context(tc.tile_pool(name="xp", bufs=3))
            apool = ctx.enter_context(tc.tile_pool(name="apool", bufs=1))
            opool = ctx.enter_context(tc.tile_pool(name="op", bufs=4))
            ps1pool = ctx.enter_context(tc.tile_pool(name="ps1", bufs=3, space="PSUM"))
            ps2pool = ctx.enter_context(tc.tile_pool(name="ps2", bufs=4, space="PSUM"))

            # Prefetch chunk 0's tokens before the weights so the first
            # matmuls can start as soon as the first wfc H-slice lands.
            x_tiles = {}
            x_tiles[0] = xpool.tile([P, KD, chunks[0]], f16, tag="xt", name="xt0")
            nc.sync.dma_start(x_tiles[0][:], xT_v[:, :, 0:chunks[0]])

            # Weights cached in SBUF for the whole kernel. wfc is split into
            # 16 H-slices: mm1's h-block mh only depends on slice mh//2
            # (subregion-granular deps), so compute starts ~1 MB in.
            wfc_sb = const.tile([P, KD, H], f16)
            for hc in range(16):
                sl = slice(hc * (H // 16), (hc + 1) * (H // 16))
                nc.sync.dma_start(wfc_sb[:, :, sl], wfcT_v[:, :, sl])
            g_sb = const.tile([P, C // P], f32)
            nc.sync.dma_start(g_sb[:], g.ap())
            # wproj is only needed once mm2 of chunk 0 starts (~55us in);
            # its load fully overlaps chunk 0's mm1.
            wproj_sb = const.tile([P, KH, D], f16)
            for kc in range(8):
                sl = slice(kc * (KH // 8), (kc + 1) * (KH // 8))
                nc.sync.dma_start(wproj_sb[:, sl, :], wprojT_v[:, sl, :])

            tok0 = 0
            for c, S in enumerate(chunks):
                if c not in x_tiles:
                    x_tiles[c] = xpool.tile([P, KD, S], f16, tag="xt", name=f"xt{c}")
                    nc.sync.dma_start(
                        x_tiles[c][:], xT_v[:, :, tok0:tok0 + S]
                    )
                x_tile = x_tiles[c]
                a_tile = apool.tile([P, KH, S], f16, tag="at")
                for mh in range(KH):
                    ps1 = ps1pool.tile([P, S], f32, tag="ps1")
                    for k in range(KD):
                        nc.tensor.matmul(
                            ps1[:],
                            wfc_sb[:, k, mh * P:(mh + 1) * P],
                            x_tile[:, k, :],
                            start=(k == 0),
                            stop=(k == KD - 1),
                        )
                    # a = prelu(h, 0.5) then a *= a  -> square(leaky_relu(h, .5))
                    nc.scalar.activation(a_tile[:, mh, :], ps1[:], AF.Prelu, alpha=0.5)
                    nc.vector.tensor_tensor(
                        a_tile[:, mh, :], a_tile[:, mh, :], a_tile[:, mh, :],
                        mybir.AluOpType.mult,
                    )
                for ti in range(S // P):
                    gcol = tok0 // P + ti
                    for dn in range(DN):
                        ps2 = ps2pool.tile([P, CHUNK], f32, tag="ps2")
                        for k in range(KH):
                            nc.tensor.matmul(
                                ps2[:],
                                a_tile[:, k, ti * P:(ti + 1) * P],
                                wproj_sb[:, k, dn * CHUNK:(dn + 1) * CHUNK],
                                start=(k == 0),
                                stop=(k == KH - 1),
                            )
                        o_tile = opool.tile([P, CHUNK], f32, tag="ot")
                        # fused gate: out = psum * g[token] (per-partition scale)
                        nc.scalar.activation(
                            o_tile[:], ps2[:], AF.Copy,
                            scale=g_sb[:, gcol:gcol + 1],
                        )
                        nc.sync.dma_start(
                            out_v[:, gcol, dn * CHUNK:(dn + 1) * CHUNK], o_tile[:]
                        )
                tok0 += S
    nc.compile()
    return nc


def _route(xf, Wg):
    """Exact top-2 gating in fp32, mirroring the reference math."""
    logits = xf @ Wg.T                                   # [N, E]
    top2 = np.argpartition(logits, E - 2, axis=1)[:, E - 2:]   # [N, 2] unordered
    vals = np.take_along_axis(logits, top2, axis=1)
    m = vals.max(axis=1, keepdims=True)
    ex = np.exp(vals - m)
    w = ex / ex.sum(axis=1, keepdims=True)               # [N, 2] softmax over top-2
    return top2, w


def run_moe(x, Wg, Wfc, Wproj, trace=False):
    from concourse import bass_utils

    xf = np.ascontiguousarray(x.reshape(-1, D), dtype=np.float32)
    top2, w = _route(xf, Wg.astype(np.float32))

    toks, gates = [], []
    for e in range(E):
        sel = np.nonzero((top2 == e).any(axis=1))[0]
        ge = (w[sel] * (top2[sel] == e)).sum(axis=1).astype(np.float32)
        toks.append(sel)
        gates.append(ge)

    maxc = max(len(t) for t in toks)
    C = max(P, ((maxc + P - 1) // P) * P)

    nc = _build_nc(C)

    xf16 = xf.astype(np.float16)
    in_maps = []
    for e in range(E):
        te = toks[e]
        xT_e = np.zeros((D, C), np.float16)
        xT_e[:, :len(te)] = xf16[te].T
        g_e = np.zeros((C,), np.float32)
        g_e[:len(te)] = gates[e]
        g_mat = np.ascontiguousarray(g_e.reshape(C // P, P).T)
        in_maps.append({
            "xT": xT_e,
            "wfcT": Wfc[e].T.astype(np.float16, order="C"),
            "wprojT": Wproj[e].T.astype(np.float16, order="C"),
            "g": g_mat,
        })

    # NTFF tracing is unavailable under this axon environment (no
    # antenv.axon_hooks); always run untraced.
    res = bass_utils.run_bass_kernel_spmd(
        nc, in_maps, core_ids=list(range(E)), trace=False
    )

    out = np.zeros((N, D), np.float32)
    for e in range(E):
        te = toks[e]
        out[te] += res.results[e]["outp"][:len(te)]
    return out.reshape(B, T, D), res


def kernel(x, Wg, Wfc, Wproj):
    out, _ = run_moe(np.asarray(x), np.asarray(Wg), np.asarray(Wfc), np.asarray(Wproj))
    return out


# revision 11
# speedup vs baseline: 1.0204x; 1.0204x over previous
"""MoE MLP (top-2 of 8 experts) Trainium2 kernel.

Strategy: expert-parallel across the 8 NeuronCores. The host computes the
(cheap, tiny) top-2 gating exactly in fp32, gathers each expert's tokens into
a contiguous capacity-padded buffer, and core e runs expert e's two big
matmuls over its gathered tokens:

    out_e = g_e * (prelu(Wfc[e] @ xT_sel, 0.5)^2)^T-chain  (all on device)

Device layout keeps the contraction dim on SBUF partitions throughout:
  mm1: psum[h_blk(128), tok(512)] += wfcT[d, h_blk]^T @ xT[d, tok]
  act: a = prelu(psum, 0.5); a *= a      (ScalarE + VectorE, fp16 out)
  mm2: psum[tok(128), d(512)]  += a[h, tok]^T @ wprojT[h, d]
  evict: out = psum * gate[token]        (per-partition scale on ScalarE)

Host scatters per-expert rows back (each token appears in exactly 2 expert
lists) and sums - identical math to the reference's dense masked combine.

Matmul inputs are fp16 (values are O(1); fp32 PSUM accumulation), weights are
cached in SBUF once per core, tokens stream through in 512-wide chunks.
"""

import numpy as np
from contextlib import ExitStack

B, T, D, H, E = 4, 2048, 1024, 4096, 8
N = B * T
P = 128
CHUNK = 512


def _build_nc(C):
    """Build + compile the per-core Bass program for capacity C tokens.

    C must be a multiple of 128. Tokens stream in chunks of 512 plus one
    optional tail chunk of C % 512.
    """
    import concourse.bacc as bacc
    import concourse.tile as tile
    import concourse.mybir as mybir

    assert C % P == 0
    f16 = mybir.dt.float16
    f32 = mybir.dt.float32
    AF = mybir.ActivationFunctionType

    nc = bacc.Bacc(None, target_bir_lowering=False, debug=False)
    xT = nc.dram_tensor("xT", [D, C], f16, kind="ExternalInput")
    wfcT = nc.dram_tensor("wfcT", [D, H], f16, kind="ExternalInput")
    wprojT = nc.dram_tensor("wprojT", [H, D], f16, kind="ExternalInput")
    g = nc.dram_tensor("g", [P, C // P], f32, kind="ExternalInput")
    out = nc.dram_tensor("outp", [C, D], f32, kind="ExternalOutput")

    xT_v = xT.ap().rearrange("(ko p) c -> p ko c", p=P)          # [128, 8, C]
    wfcT_v = wfcT.ap().rearrange("(ko p) h -> p ko h", p=P)      # [128, 8, H]
    wprojT_v = wprojT.ap().rearrange("(ko p) d -> p ko d", p=P)  # [128, 32, D]
    out_v = out.ap().rearrange("(c p) d -> p c d", p=P)          # [128, C//128, D]

    KD = D // P          # 8  k-subtiles for mm1
    KH = H // P          # 32 k-subtiles for mm2 (and h-blocks of mm1 output)
    DN = D // CHUNK      # 2 output-column blocks

    # Full chunks first, tail last: chunk 0's mm1 pace (~1.7us per h-block)
    # matches the wfc SBUF fill rate, so the PE never stalls on weights.
    chunks = [CHUNK] * (C // CHUNK)
    if C % CHUNK:
        chunks.append(C % CHUNK)

    with tile.TileContext(nc) as tc:
        with ExitStack() as ctx:
            const = ctx.enter_context(tc.tile_pool(name="const", bufs=1))
            xpool = ctx.enter_context(tc.tile_pool(name="xp", bufs=3))
            apool = ctx.enter_context(tc.tile_pool(name="apool", bufs=1))
            opool = ctx.enter_context(tc.tile_pool(name="op", bufs=4))
            ps1pool = ctx.enter_context(tc.tile_pool(name="ps1", bufs=4, space="PSUM"))
            ps2pool = ctx.enter_context(tc.tile_pool(name="ps2", bufs=4, space="PSUM"))

            # Startup-critical DMAs first, each on its own round-robin queue:
            # the first matmul (mh=0, k=0) waits only on wfc cols 0:128
            # (256 KB) and x chunk-0 k-slice 0 (128 KB), loading in parallel.
            x_tiles = {}
            x_tiles[0] = xpool.tile([P, KD, chunks[0]], f16, tag="xt", name="xt0")
            wfc_sb = const.tile([P, KD, H], f16)
            nc.sync.dma_start(wfc_sb[:, :, 0:P], wfcT_v[:, :, 0:P])
            nc.sync.dma_start(x_tiles[0][:, 0, :], xT_v[:, 0, 0:chunks[0]])
            nc.sync.dma_start(wfc_sb[:, :, P:2 * P], wfcT_v[:, :, P:2 * P])
            for k in range(1, KD):
                nc.sync.dma_start(
                    x_tiles[0][:, k, :], xT_v[:, k, 0:chunks[0]]
                )
            # Rest of wfc in H-slices: mm1's h-block mh only depends on the
            # slices covering its 128 columns (subregion-granular deps).
            for s0 in range(2 * P, H, H // 16):
                w = H // 16
                nc.sync.dma_start(
                    wfc_sb[:, :, s0:s0 + w], wfcT_v[:, :, s0:s0 + w]
                )
            # wproj is only needed once mm2 of chunk 0 starts (~55us in);
            # its load fully overlaps chunk 0's mm1. Same for g.
            wproj_sb = const.tile([P, KH, D], f16)
            for kc in range(8):
                sl = slice(kc * (KH // 8), (kc + 1) * (KH // 8))
                nc.sync.dma_start(wproj_sb[:, sl, :], wprojT_v[:, sl, :])
            g_sb = const.tile([P, C // P], f32)
            nc.sync.dma_start(g_sb[:], g.ap())

            tok0 = 0
            for c, S in enumerate(chunks):
                if c not in x_tiles:
                    x_tiles[c] = xpool.tile([P, KD, S], f16, tag="xt", name=f"xt{c}")
                    nc.sync.dma_start(
                        x_tiles[c][:], xT_v[:, :, tok0:tok0 + S]
                    )
                x_tile = x_tiles[c]
                a_tile = apool.tile([P, KH, S], f16, tag="at")
                for mh in range(KH):
                    ps1 = ps1pool.tile([P, S], f32, tag="ps1")
                    for k in range(KD):
                        nc.tensor.matmul(
                            ps1[:],
                            wfc_sb[:, k, mh * P:(mh + 1) * P],
                            x_tile[:, k, :],
                            start=(k == 0),
                            stop=(k == KD - 1),
                        )
                    # a = prelu(h, 0.5) then a *= a  -> square(leaky_relu(h, .5))
                    nc.scalar.activation(a_tile[:, mh, :], ps1[:], AF.Prelu, alpha=0.5)
                    nc.vector.tensor_tensor(
                        a_tile[:, mh, :], a_tile[:, mh, :], a_tile[:, mh, :],
                        mybir.AluOpType.mult,
                    )
                for ti in range(S // P):
                    gcol = tok0 // P + ti
                    for dn in range(DN):
                        ps2 = ps2pool.tile([P, CHUNK], f32, tag="ps2")
                        for k in range(KH):
                            nc.tensor.matmul(
                                ps2[:],
                                a_tile[:, k, ti * P:(ti + 1) * P],
                                wproj_sb[:, k, dn * CHUNK:(dn + 1) * CHUNK],
                                start=(k == 0),
                                stop=(k == KH - 1),
                            )
                        o_tile = opool.tile([P, CHUNK], f32, tag="ot")
                        # fused gate: out = psum * g[token] (per-partition scale)
                        nc.scalar.activation(
                            o_tile[:], ps2[:], AF.Copy,
                            scale=g_sb[:, gcol:gcol + 1],
                        )
                        nc.sync.dma_start(
                            out_v[:, gcol, dn * CHUNK:(dn + 1) * CHUNK], o_tile[:]
                        )
                tok0 += S
    nc.compile()
    return nc


def _route(xf, Wg):
    """Exact top-2 gating in fp32, mirroring the reference math."""
    logits = xf @ Wg.T                                   # [N, E]
    top2 = np.argpartition(logits, E - 2, axis=1)[:, E - 2:]   # [N, 2] unordered
    vals = np.take_along_axis(logits, top2, axis=1)
    m = vals.max(axis=1, keepdims=True)
    ex = np.exp(vals - m)
    w = ex / ex.sum(axis=1, keepdims=True)               # [N, 2] softmax over top-2
    return top2, w


def run_moe(x, Wg, Wfc, Wproj, trace=False):
    from concourse import bass_utils

    xf = np.ascontiguousarray(x.reshape(-1, D), dtype=np.float32)
    top2, w = _route(xf, Wg.astype(np.float32))

    toks, gates = [], []
    for e in range(E):
        sel = np.nonzero((top2 == e).any(axis=1))[0]
        ge = (w[sel] * (top2[sel] == e)).sum(axis=1).astype(np.float32)
        toks.append(sel)
        gates.append(ge)

    maxc = max(len(t) for t in toks)
    C = max(P, ((maxc + P - 1) // P) * P)

    nc = _build_nc(C)

    xf16 = xf.astype(np.float16)
    in_maps = []
    for e in range(E):
        te = toks[e]
        xT_e = np.zeros((D, C), np.float16)
        xT_e[:, :len(te)] = xf16[te].T
        g_e = np.zeros((C,), np.float32)
        g_e[:len(te)] = gates[e]
        g_mat = np.ascontiguousarray(g_e.reshape(C // P, P).T)
        in_maps.append({
            "xT": xT_e,
            "wfcT": Wfc[e].T.astype(np.float16, order="C"),
            "wprojT": Wproj[e].T.astype(np.float16, order="C"),
            "g": g_mat,
        })

    # NTFF tracing is unavailable under this axon environment (no
    # antenv.axon_hooks); always run untraced.
    res = bass_utils.run_bass_kernel_spmd(
        nc, in_maps, core_ids=list(range(E)), trace=False
    )

    out = np.zeros((N, D), np.float32)
    for e in range(E):
        te = toks[e]
        out[te] += res.results[e]["outp"][:len(te)]
    return out.reshape(B, T, D), res


def kernel(x, Wg, Wfc, Wproj):
    out, _ = run_moe(np.asarray(x), np.asarray(Wg), np.asarray(Wfc), np.asarray(Wproj))
    return out


# revision 20
# speedup vs baseline: 1.0231x; 1.0027x over previous
"""MoE MLP (top-2 of 8 experts) Trainium2 kernel.

Strategy: expert-parallel across the 8 NeuronCores. The host computes the
(cheap, tiny) top-2 gating exactly in fp32, gathers each expert's tokens into
a contiguous capacity-padded buffer, and core e runs expert e's two big
matmuls over its gathered tokens:

    out_e = g_e * (prelu(Wfc[e] @ xT_sel, 0.5)^2)^T-chain  (all on device)

Device layout keeps the contraction dim on SBUF partitions throughout:
  mm1: psum[h_blk(128), tok(512)] += wfcT[d, h_blk]^T @ xT[d, tok]
  act: a = prelu(psum, 0.5); a *= a      (ScalarE + VectorE, fp16 out)
  mm2: psum[tok(128), d(512)]  += a[h, tok]^T @ wprojT[h, d]
  evict: out = psum * gate[token]        (per-partition scale on ScalarE)

Host scatters per-expert rows back (each token appears in exactly 2 expert
lists) and sums - identical math to the reference's dense masked combine.

Matmul inputs are fp16 (values are O(1); fp32 PSUM accumulation), weights are
cached in SBUF once per core, tokens stream through in 512-wide chunks.
"""

import numpy as np
from contextlib import ExitStack

B, T, D, H, E = 4, 2048, 1024, 4096, 8
N = B * T
P = 128
CHUNK = 512


def _build_nc(C):
    """Build + compile the per-core Bass program for capacity C tokens.

    C must be a multiple of 128. Tokens stream in chunks of 512 plus one
    optional tail chunk of C % 512.
    """
    import concourse.bacc as bacc
    import concourse.tile as tile
    import concourse.mybir as mybir

    assert C % P == 0
    f16 = mybir.dt.float16
    f32 = mybir.dt.float32
    AF = mybir.ActivationFunctionType

    nc = bacc.Bacc(None, target_bir_lowering=False, debug=False)
    xT = nc.dram_tensor("xT", [D, C], f16, kind="ExternalInput")
    wfcT = nc.dram_tensor("wfcT", [D, H], f16, kind="ExternalInput")
    wprojT = nc.dram_tensor("wprojT", [H, D], f16, kind="ExternalInput")
    g = nc.dram_tensor("g", [P, C // P], f32, kind="ExternalInput")
    out = nc.dram_tensor("outp", [C, D], f32, kind="ExternalOutput")

    xT_v = xT.ap().rearrange("(ko p) c -> p ko c", p=P)          # [128, 8, C]
    wfcT_v = wfcT.ap().rearrange("(ko p) h -> p ko h", p=P)      # [128, 8, H]
    wprojT_v = wprojT.ap().rearrange("(ko p) d -> p ko d", p=P)  # [128, 32, D]
    out_v = out.ap().rearrange("(c p) d -> p c d", p=P)          # [128, C//128, D]

    KD = D // P          # 8  k-subtiles for mm1
    KH = H // P          # 32 k-subtiles for mm2 (and h-blocks of mm1 output)
    DN = D // CHUNK      # 2 output-column blocks

    # Full chunks first, tail last: chunk 0's mm1 pace (~1.7us per h-block)
    # matches the wfc SBUF fill rate, so the PE never stalls on weights.
    chunks = [CHUNK] * (C // CHUNK)
    if C % CHUNK:
        chunks.append(C % CHUNK)

    with tile.TileContext(nc) as tc:
        with ExitStack() as ctx:
            const = ctx.enter_context(tc.tile_pool(name="const", bufs=1))
            xpool = ctx.enter_context(tc.tile_pool(name="xp", bufs=3))
            apool = ctx.enter_context(tc.tile_pool(name="apool", bufs=1))
            opool = ctx.enter_context(tc.tile_pool(name="op", bufs=4))
            ps1pool = ctx.enter_context(tc.tile_pool(name="ps1", bufs=3, space="PSUM"))
            ps2pool = ctx.enter_context(tc.tile_pool(name="ps2", bufs=4, space="PSUM"))
            warmpool = ctx.enter_context(tc.tile_pool(name="wm", bufs=1, space="PSUM"))

            # Startup-critical DMAs first, each on its own round-robin queue:
            # the first matmul (mh=0, k=0) waits only on wfc cols 0:128
            # (256 KB) and x chunk-0 k-slice 0 (128 KB), loading in parallel.
            x_tiles = {}
            x_tiles[0] = xpool.tile([P, KD, chunks[0]], f16, tag="xt", name="xt0")
            wfc_sb = const.tile([P, KD, H], f16)
            nc.sync.dma_start(wfc_sb[:, :, 0:P], wfcT_v[:, :, 0:P])
            nc.sync.dma_start(x_tiles[0][:, 0:2, :], xT_v[:, 0:2, 0:chunks[0]])
            nc.sync.dma_start(wfc_sb[:, :, P:2 * P], wfcT_v[:, :, P:2 * P])
            nc.sync.dma_start(x_tiles[0][:, 2:5, :], xT_v[:, 2:5, 0:chunks[0]])
            nc.sync.dma_start(x_tiles[0][:, 5:KD, :], xT_v[:, 5:KD, 0:chunks[0]])
            # Rest of wfc in H-slices: mm1's h-block mh only depends on the
            # slices covering its 128 columns (subregion-granular deps).
            for s0 in range(2 * P, H, H // 16):
                w = H // 16
                nc.sync.dma_start(
                    wfc_sb[:, :, s0:s0 + w], wfcT_v[:, :, s0:s0 + w]
                )
            # wproj is only needed once mm2 of chunk 0 starts (~55us in);
            # its load fully overlaps chunk 0's mm1. Same for g.
            wproj_sb = const.tile([P, KH, D], f16)
            for kc in range(8):
                sl = slice(kc * (KH // 8), (kc + 1) * (KH // 8))
                nc.sync.dma_start(wproj_sb[:, sl, :], wprojT_v[:, sl, :])
            g_sb = const.tile([P, C // P], f32)
            nc.sync.dma_start(g_sb[:], g.ap())

            # PE warmup: the HAM clock-gate needs ~3.4us of sustained matmul
            # activity to grant the 2.4 GHz rate. The PE is idle waiting for
            # the first DMAs anyway, so burn that window on dummy matmuls
            # over a zeroed scratch tile (results never read).
            warm_sb = const.tile([P, P], f16)
            nc.vector.memset(warm_sb[:], 0.0)
            warm_ps = warmpool.tile([P, P], f32)
            for _ in range(38):
                nc.tensor.matmul(warm_ps[:], warm_sb[:], warm_sb[:],
                                 start=True, stop=True)

            tok0 = 0
            for c, S in enumerate(chunks):
                if c not in x_tiles:
                    x_tiles[c] = xpool.tile([P, KD, S], f16, tag="xt", name=f"xt{c}")
                    nc.sync.dma_start(
                        x_tiles[c][:], xT_v[:, :, tok0:tok0 + S]
                    )
                x_tile = x_tiles[c]
                a_tile = apool.tile([P, KH, S], f16, tag="at")
                for mh in range(KH):
                    ps1 = ps1pool.tile([P, S], f32, tag="ps1")
                    for k in range(KD):
                        nc.tensor.matmul(
                            ps1[:],
                            wfc_sb[:, k, mh * P:(mh + 1) * P],
                            x_tile[:, k, :],
                            start=(k == 0),
                            stop=(k == KD - 1),
                        )
                    # a = prelu(h, 0.5) then a *= a  -> square(leaky_relu(h, .5))
                    nc.scalar.activation(a_tile[:, mh, :], ps1[:], AF.Prelu, alpha=0.5)
                    nc.vector.tensor_tensor(
                        a_tile[:, mh, :], a_tile[:, mh, :], a_tile[:, mh, :],
                        mybir.AluOpType.mult,
                    )
                for ti in range(S // P):
                    gcol = tok0 // P + ti
                    for dn in range(DN):
                        ps2 = ps2pool.tile([P, CHUNK], f32, tag="ps2")
                        for k in range(KH):
                            nc.tensor.matmul(
                                ps2[:],
                                a_tile[:, k, ti * P:(ti + 1) * P],
                                wproj_sb[:, k, dn * CHUNK:(dn + 1) * CHUNK],
                                start=(k == 0),
                                stop=(k == KH - 1),
                            )
                        o_tile = opool.tile([P, CHUNK], f32, tag="ot")
                        # fused gate: out = psum * g[token] (per-partition scale)
                        nc.scalar.activation(
                            o_tile[:], ps2[:], AF.Copy,
                            scale=g_sb[:, gcol:gcol + 1],
                        )
                        nc.sync.dma_start(
                            out_v[:, gcol, dn * CHUNK:(dn + 1) * CHUNK], o_tile[:]
                        )
                tok0 += S
    nc.compile()
    return nc


def _route(xf, Wg):
    """Exact top-2 gating in fp32, mirroring the reference math."""
    logits = xf @ Wg.T                                   # [N, E]
    top2 = np.argpartition(logits, E - 2, axis=1)[:, E - 2:]   # [N, 2] unordered
    vals = np.take_along_axis(logits, top2, axis=1)
    m = vals.max(axis=1, keepdims=True)
    ex = np.exp(vals - m)
    w = ex / ex.sum(axis=1, keepdims=True)               # [N, 2] softmax over top-2
    return top2, w


def run_moe(x, Wg, Wfc, Wproj, trace=False):
    from concourse import bass_utils

    xf = np.ascontiguousarray(x.reshape(-1, D), dtype=np.float32)
    top2, w = _route(xf, Wg.astype(np.float32))

    toks, gates = [], []
    for e in range(E):
        sel = np.nonzero((top2 == e).any(axis=1))[0]
        ge = (w[sel] * (top2[sel] == e)).sum(axis=1).astype(np.float32)
        toks.append(sel)
        gates.append(ge)

    maxc = max(len(t) for t in toks)
    C = max(P, ((maxc + P - 1) // P) * P)

    nc = _build_nc(C)

    xf16 = xf.astype(np.float16)
    in_maps = []
    for e in range(E):
        te = toks[e]
        xT_e = np.zeros((D, C), np.float16)
        xT_e[:, :len(te)] = xf16[te].T
        g_e = np.zeros((C,), np.float32)
        g_e[:len(te)] = gates[e]
        g_mat = np.ascontiguousarray(g_e.reshape(C // P, P).T)
        in_maps.append({
            "xT": xT_e,
            "wfcT": Wfc[e].T.astype(np.float16, order="C"),
            "wprojT": Wproj[e].T.astype(np.float16, order="C"),
            "g": g_mat,
        })

    # NTFF tracing is unavailable under this axon environment (no
    # antenv.axon_hooks); always run untraced.
    res = bass_utils.run_bass_kernel_spmd(
        nc, in_maps, core_ids=list(range(E)), trace=False
    )

    out = np.zeros((N, D), np.float32)
    for e in range(E):
        te = toks[e]
        out[te] += res.results[e]["outp"][:len(te)]
    return out.reshape(B, T, D), res


def kernel(x, Wg, Wfc, Wproj):
    out, _ = run_moe(np.asarray(x), np.asarray(Wg), np.asarray(Wfc), np.asarray(Wproj))
    return out
